# revision 15
# baseline (speedup 1.0000x reference)
"""MoE (top-2 of 16 experts, SwiGLU MLP) kernel for 8 Trainium2 NeuronCores.

Two-tier expert-parallel strategy:
  - Host router (fp64) gathers tokens per expert. Within each expert the
    highest-gate tokens run in bf16; the low-gate tail runs entirely in
    fp8-e4m3 with DoubleRow matmuls (2x PE rate). Low gates scale the
    fp8 error down so the global rel err stays ~1.6e-2 (< 2e-2 gate).
  - bf16 tier: 8 biggest experts get b0=824 tokens (slot 0), the rest
    b1=688 (slot 1) -- zero padding in the bf16 tier. fp8 batches are
    ranked by size into two slots with uniform caps.
  - Device math per batch:
        ht = silu(W1.T @ Xt) * (W2.T @ Xt);  yt = (WC.T @ ht) * gate
    fp8: x,w scaled on host (sx=0.6, sw=6, swc=512); silu descales via
    the activation input scale; h is cast to fp8 at scale sx*sw; the
    final descale is folded into the gate row.
  - Host: scatter-add per-batch outputs back to token order (fp32).
"""

import contextlib
import ctypes
import os
import sys
import types

sys.path.insert(0, "/opt/trn_rl_repo")

import numpy as np
import ml_dtypes

import concourse.bass as bass
import concourse.mybir as mybir
import concourse.tile as tile

BF16 = ml_dtypes.bfloat16
FP8 = ml_dtypes.float8_e4m3  # matches mybir.dt.float8e4 (max 240)
EMB = 1024
HID = 1024
E = 16
TOPK = 2
NCORES = 8
EPC = E // NCORES  # bf16 experts per core
P = 128
KT = EMB // P  # contraction tiles per GEMM (8)
HT = HID // P  # hidden/output row-blocks (8)

B0, B1 = 776, 640  # bf16 tokens per slot-0 / slot-1 expert
SX, SW, SWC = 0.6, 6.0, 512.0  # fp8 scales; h scale = SX*SW
SH = SX * SW


def _install_profile_shim():
    """Register the axon NTFF profiling hook (missing antenv.axon_hooks in
    this image) so run_bass_kernel_spmd(trace=True) can measure HW time."""
    if "antenv.axon_hooks" in sys.modules:
        return
    try:
        lib = ctypes.CDLL("/opt/axon/libaxon_pjrt.so")
        lib.axon_start_nrt_profile.argtypes = [
            ctypes.POINTER(ctypes.c_int64),
            ctypes.c_size_t,
        ]
        lib.axon_start_nrt_profile.restype = ctypes.c_int64
        lib.axon_stop_nrt_profile.argtypes = [ctypes.c_char_p]
        lib.axon_stop_nrt_profile.restype = ctypes.c_int64
    except Exception:
        return

    @contextlib.contextmanager
    def _hook(output_dir, device_ids):
        import jax

        jax.devices()
        ids = (
            (ctypes.c_int64 * len(device_ids))(*device_ids) if device_ids else None
        )
        rc = lib.axon_start_nrt_profile(ids, len(device_ids) if device_ids else 0)
        if rc != 0:
            raise RuntimeError(f"axon_start_nrt_profile rc={rc}")
        try:
            yield
        finally:
            n = lib.axon_stop_nrt_profile(str(output_dir).encode())
            print(f"profile: {n} file(s) written to {output_dir}")

    mod = types.ModuleType("antenv.axon_hooks")
    mod.get_axon_ntff_profile_hook = lambda: _hook
    mod.set_axon_ntff_profile_hook = lambda h: None
    sys.modules["antenv.axon_hooks"] = mod


def _split_multi_waits(nc):
    """This container's walrus only encodes one sem wait per CTRL-class
    instruction; hoist extra waits onto dedicated single-wait NoOps."""
    idx = 0
    for fn in nc.m.functions:
        for bb in fn.blocks:
            new = []
            for inst in bb.instructions:
                si = inst.sync_info
                if si is not None and len(si.on_wait) > 1:
                    waits = list(si.on_wait)
                    for w in waits[:-1]:
                        c = mybir.InstNoOp(name=f"wsplit-{idx}", ins=[], outs=[])
                        idx += 1
                        c.engine = inst.engine
                        c.sync_info = mybir.SyncInfo(on_wait=[w], on_update=[])
                        new.append(c)
                    si.on_wait = [waits[-1]]
                new.append(inst)
            bb.instructions = new


def _token_chunks(C):
    """Split C (multiple of 4) into a 320-col lead chunk (fast DMA -> PE
    starts early; paces the first h-pass to the weight feed) plus
    near-equal chunks <=512, multiples of 4."""
    lead = min(320, C)
    R = C - lead
    if R == 0:
        return [lead]
    n = max(1, -(-R // 512))
    base = R // n
    base -= base % 4
    sizes = [base] * n
    rem = R - base * n
    i = 0
    while rem > 0:
        add = min(4, rem)
        sizes[i] += add
        rem -= add
        i = (i + 1) % n
    return [lead] + [s for s in sizes if s > 0]


def _build_bass(caps, fcaps):
    F32 = mybir.dt.float32
    B16 = mybir.dt.bfloat16
    F8 = mybir.dt.float8e4
    DR = mybir.MatmulPerfMode.DoubleRow

    nc = bass.Bass()
    xt_d, g_d, w12_d, wc_d, yt_d = [], [], [], [], []
    for e in range(EPC):
        C = caps[e]
        xt_d.append(nc.declare_dram_parameter(f"xt{e}", [P, KT * C], B16, isOutput=False))
        g_d.append(nc.declare_dram_parameter(f"g{e}", [P, C], F32, isOutput=False))
        w12_d.append(
            nc.declare_dram_parameter(
                f"w12_{e}", [P, HT * 2 * KT * P], B16, isOutput=False
            )
        )
        wc_d.append(
            nc.declare_dram_parameter(f"wc_{e}", [P, HT * HT * P], B16, isOutput=False)
        )
        yt_d.append(nc.declare_dram_parameter(f"yt{e}", [P, HT * C], B16, isOutput=True))
    x8_d, g8_d, w128_d, wc8_d, yt8_d = [], [], [], [], []
    for f in range(EPC):
        Fc = fcaps[f]
        x8_d.append(nc.declare_dram_parameter(f"x8_{f}", [P, KT * Fc], F8, isOutput=False))
        g8_d.append(nc.declare_dram_parameter(f"g8_{f}", [P, Fc], F32, isOutput=False))
        w128_d.append(
            nc.declare_dram_parameter(
                f"w128_{f}", [P, HT * 2 * KT * P], F8, isOutput=False
            )
        )
        wc8_d.append(
            nc.declare_dram_parameter(f"wc8_{f}", [P, HT * HT * P], F8, isOutput=False)
        )
        yt8_d.append(
            nc.declare_dram_parameter(f"yt8_{f}", [P, HT * Fc], B16, isOutput=True)
        )

    with tile.TileContext(nc) as tc:
        with (
            tc.tile_pool(name="xt", bufs=4) as xt_pool,
            tc.tile_pool(name="x8", bufs=2) as x8_pool,
            tc.tile_pool(name="ht", bufs=2) as ht_pool,
            tc.tile_pool(name="h8", bufs=2) as h8_pool,
            tc.tile_pool(name="g", bufs=2) as g_pool,
            tc.tile_pool(name="g8", bufs=2) as g8_pool,
            tc.tile_pool(name="w12", bufs=10) as w12_pool,
            tc.tile_pool(name="w128", bufs=17) as w128_pool,
            tc.tile_pool(name="wc", bufs=8) as wc_pool,
            tc.tile_pool(name="wc8", bufs=16) as wc8_pool,
            tc.tile_pool(name="s", bufs=3) as s_pool,
            tc.tile_pool(name="y", bufs=3) as y_pool,
            tc.tile_pool(name="psA", bufs=2, space="PSUM") as psA,
            tc.tile_pool(name="psB", bufs=3, space="PSUM") as psB,
            tc.tile_pool(name="psW", bufs=1, space="PSUM") as psW,
        ):
            # PE p-state warm-up during the initial DMA wait.
            warm_in = s_pool.tile([P, 256], mybir.dt.bfloat16, tag="warm")
            nc.vector.memset(warm_in[:], 0.0)
            warm_ps = psW.tile([P, 256], mybir.dt.float32, tag="warmps")
            for _ in range(20):
                nc.tensor.matmul(
                    warm_ps[:], warm_in[:, 0:P], warm_in[:], start=True, stop=True
                )

            all_chunks = [_token_chunks(caps[e]) for e in range(EPC)]
            w12t = [[None] * HT for _ in range(EPC)]
            w128t = [[None] * HT for _ in range(EPC)]
            xt_t = [[] for _ in range(EPC)]
            x8_t = [None] * EPC
            g_sb = [None] * EPC
            g8_sb = [None] * EPC
            WB = 2 * KT * P  # w12 elements per h-block

            # ---- Hoisted input DMA issues (in-order sync engine: these
            # must never queue behind semaphore-gated y-DMA issues).
            # Priority: e0 weights h0 + lead chunk, e0 rest, e1 critical
            # prefix, fp8 f0 inputs, gates. The remainder is issued after
            # e0's phase A (pool buffers free up by then).
            def issue_w12(e, h, eng=None):
                w12t[e][h] = w12_pool.tile(
                    [P, 2, KT, P], B16, tag="w12", name=f"w12_{e}_{h}"
                )
                (eng or nc.sync).dma_start(
                    w12t[e][h][:],
                    w12_d[e][:, h * WB : (h + 1) * WB].rearrange(
                        "p (t k q) -> p t k q", t=2, k=KT
                    ),
                )

            def issue_xt(e, ci, c0, cs, eng=None):
                t = xt_pool.tile([P, KT, cs], B16, tag="xt", name=f"xt_{e}_{ci}")
                xt_t[e].append(t)
                (eng or nc.sync).dma_start(
                    t[:],
                    xt_d[e][:, KT * c0 : KT * (c0 + cs)].rearrange(
                        "p (k c) -> p k c", k=KT
                    ),
                )

            def issue_w128(f, h):
                w128t[f][h] = w128_pool.tile(
                    [P, 2, KT, P], F8, tag="w128", name=f"w128_{f}_{h}"
                )
                nc.sync.dma_start(
                    w128t[f][h][:],
                    w128_d[f][:, h * WB : (h + 1) * WB].rearrange(
                        "p (t k q) -> p t k q", t=2, k=KT
                    ),
                )

            def issue_x8(f):
                x8_t[f] = x8_pool.tile(
                    [P, KT, fcaps[f]], F8, tag="x8", name=f"x8_{f}"
                )
                nc.sync.dma_start(
                    x8_t[f][:],
                    x8_d[f].rearrange("p (k c) -> p k c", k=KT),
                )

            # e0 critical path -- first issues fan out across engines so
            # the serial ~0.7us-per-issue cost is paid in parallel
            ch0 = all_chunks[0]
            issue_w12(0, 0)                      # sync
            issue_xt(0, 0, 0, ch0[0], nc.scalar)
            issue_w12(0, 1, nc.gpsimd)
            issue_w12(0, 2, nc.gpsimd)
            issue_w12(0, 3)                      # sync
            issue_w12(0, 4, nc.scalar)
            issue_w12(0, 5, nc.gpsimd)
            issue_w12(0, 6)                      # sync
            issue_w12(0, 7, nc.scalar)
            c0 = ch0[0]
            for ci in range(1, len(ch0)):
                issue_xt(0, ci, c0, ch0[ci], nc.gpsimd)
                c0 += ch0[ci]
            # gates + e0's wc (needed when e0 phase B starts)
            def issue_g(e):
                g_sb[e] = g_pool.tile([P, caps[e]], F32, tag="g", name=f"g_{e}")
                nc.sync.dma_start(g_sb[e][:], g_d[e][:])

            def issue_g8(f):
                g8_sb[f] = g8_pool.tile(
                    [P, fcaps[f]], F32, tag="g8", name=f"g8_{f}"
                )
                nc.sync.dma_start(g8_sb[f][:], g8_d[f][:])

            def phase_a_bf16(e):
                C = caps[e]
                chunks = all_chunks[e]
                ht_sb = ht_pool.tile([P, HT, C], B16, tag="ht", name=f"ht_{e}")
                c0 = 0
                for ci, cs in enumerate(chunks):
                    for h in range(HT):
                        ps1 = psA.tile([P, cs], mybir.dt.float32, tag="ps1")
                        ps2 = psA.tile([P, cs], mybir.dt.float32, tag="ps2")
                        for k in range(KT):
                            nc.tensor.matmul(
                                ps1[:],
                                w12t[e][h][:, 0, k, :],
                                xt_t[e][ci][:, k, :],
                                start=(k == 0),
                                stop=(k == KT - 1),
                            )
                        for k in range(KT):
                            nc.tensor.matmul(
                                ps2[:],
                                w12t[e][h][:, 1, k, :],
                                xt_t[e][ci][:, k, :],
                                start=(k == 0),
                                stop=(k == KT - 1),
                            )
                        s_sb = s_pool.tile([P, 512], mybir.dt.float32, tag="s")
                        nc.scalar.activation(
                            s_sb[:, :cs],
                            ps1[:],
                            mybir.ActivationFunctionType.Silu,
                        )
                        nc.vector.tensor_mul(
                            ht_sb[:, h, c0 : c0 + cs], s_sb[:, :cs], ps2[:]
                        )
                    c0 += cs
                return ht_sb

            def issue_wcts(e):
                wcts = []
                for d in range(HT):
                    wct = wc_pool.tile(
                        [P, HT, P], B16, tag="wc", name=f"wc_{e}_{d}"
                    )
                    wcts.append(wct)
                    nc.sync.dma_start(
                        wct[:],
                        wc_d[e][:, d * HT * P : (d + 1) * HT * P].rearrange(
                            "p (h q) -> p h q", h=HT
                        ),
                    )
                return wcts

            def issue_wc8s(f):
                wcts = []
                for d in range(HT):
                    wct = wc8_pool.tile(
                        [P, HT, P], F8, tag="wc8", name=f"wc8_{f}_{d}"
                    )
                    wcts.append(wct)
                    nc.sync.dma_start(
                        wct[:],
                        wc8_d[f][:, d * HT * P : (d + 1) * HT * P].rearrange(
                            "p (h q) -> p h q", h=HT
                        ),
                    )
                return wcts

            def phase_b_bf16(e, ht_sb, wcts):
                chunks = all_chunks[e]
                offs = []
                c0 = 0
                for cs in chunks:
                    offs.append((c0, cs))
                    c0 += cs
                # natural order: lead chunk first (its ht was finished
                # earliest in phase A, so B never waits on the DVE tail)
                yt_view = yt_d[e].rearrange("p (d c) -> p d c", d=HT)
                for d in range(HT):
                    for c0, cs in offs:
                        psy = psB.tile([P, cs], mybir.dt.float32, tag="psy")
                        for h in range(HT):
                            nc.tensor.matmul(
                                psy[:],
                                wcts[d][:, h, :],
                                ht_sb[:, h, c0 : c0 + cs],
                                start=(h == 0),
                                stop=(h == HT - 1),
                            )
                        y_sb = y_pool.tile([P, 512], B16, tag="y")
                        nc.vector.tensor_mul(
                            y_sb[:, :cs], psy[:], g_sb[e][:, c0 : c0 + cs]
                        )
                        nc.scalar.dma_start(
                            yt_view[:, d, c0 : c0 + cs],
                            y_sb[:, :cs],
                        )

            def fp8_phase_a(f):
                Fc = fcaps[f]
                h8_sb = h8_pool.tile([P, HT, Fc], F8, tag="h8", name=f"h8_{f}")
                # Phase A: 4 DoubleRow matmuls per GEMM per h-block
                for h in range(HT):
                    ps1 = psA.tile([P, Fc], mybir.dt.float32, tag="ps1")
                    ps2 = psA.tile([P, Fc], mybir.dt.float32, tag="ps2")
                    for kp in range(KT // 2):
                        nc.tensor.matmul(
                            ps1[:],
                            w128t[f][h][:, 0, 2 * kp : 2 * kp + 2, :],
                            x8_t[f][:, 2 * kp : 2 * kp + 2, :],
                            start=(kp == 0),
                            stop=(kp == KT // 2 - 1),
                            perf_mode=DR,
                        )
                    for kp in range(KT // 2):
                        nc.tensor.matmul(
                            ps2[:],
                            w128t[f][h][:, 1, 2 * kp : 2 * kp + 2, :],
                            x8_t[f][:, 2 * kp : 2 * kp + 2, :],
                            start=(kp == 0),
                            stop=(kp == KT // 2 - 1),
                            perf_mode=DR,
                        )
                    s_sb = s_pool.tile([P, 512], mybir.dt.float32, tag="s")
                    nc.scalar.activation(
                        s_sb[:, :Fc],
                        ps1[:],
                        mybir.ActivationFunctionType.Silu,
                        scale=1.0 / (SX * SW),
                    )
                    # h8 = silu(a) * b_psum  -> scale sx*sw, cast to fp8
                    nc.vector.tensor_mul(h8_sb[:, h, :], s_sb[:, :Fc], ps2[:])
                return h8_sb

            def fp8_phase_b(f, h8_sb, wcts):
                Fc = fcaps[f]
                yt_view = yt8_d[f].rearrange("p (d c) -> p d c", d=HT)
                for d in range(HT):
                    psy = psB.tile([P, Fc], mybir.dt.float32, tag="psy")
                    for hp in range(HT // 2):
                        nc.tensor.matmul(
                            psy[:],
                            wcts[d][:, 2 * hp : 2 * hp + 2, :],
                            h8_sb[:, 2 * hp : 2 * hp + 2, :],
                            start=(hp == 0),
                            stop=(hp == HT // 2 - 1),
                            perf_mode=DR,
                        )
                    y_sb = y_pool.tile([P, 512], B16, tag="y")
                    nc.vector.tensor_mul(y_sb[:, :Fc], psy[:], g8_sb[f][:])
                    nc.gpsimd.dma_start(yt_view[:, d, :], y_sb[:, :Fc])

            # remaining hoisted issues, in consumption order
            issue_g(0)
            wcts0 = issue_wcts(0)
            issue_w12(1, 0)
            issue_xt(1, 0, 0, all_chunks[1][0])
            issue_w12(1, 1)
            for h in range(HT):
                issue_w128(0, h)
            issue_x8(0)
            issue_g(1)
            issue_g8(0)
            issue_g8(1)

            # ---- Compute: e0 bf16, e1 bf16, then fp8 A/A/B/B ----
            ht0 = phase_a_bf16(0)
            # deferred input issues (buffers free during e0 phase A)
            for h in range(2, HT):
                issue_w12(1, h)
            c0 = all_chunks[1][0]
            for ci in range(1, len(all_chunks[1])):
                issue_xt(1, ci, c0, all_chunks[1][ci])
                c0 += all_chunks[1][ci]
            for h in range(HT):
                issue_w128(1, h)
            issue_x8(1)
            phase_b_bf16(0, ht0, wcts0)
            ht1 = phase_a_bf16(1)
            wcts1 = issue_wcts(1)
            wc8s0 = issue_wc8s(0)
            wc8s1 = issue_wc8s(1)
            phase_b_bf16(1, ht1, wcts1)
            h8_0 = fp8_phase_a(0)
            h8_1 = fp8_phase_a(1)
            fp8_phase_b(0, h8_0, wc8s0)
            fp8_phase_b(1, h8_1, wc8s1)

    _split_multi_waits(nc)
    return nc


def _pack_w12(wa, wb, dtype, scale=1.0):
    """[EMB, HID] x2 -> [P, HT*2*KT*P] with layout [p, h, {a,b}, k, q]."""
    ta = (wa * scale).astype(dtype).reshape(KT, P, HT, P).transpose(1, 2, 0, 3)
    tb = (wb * scale).astype(dtype).reshape(KT, P, HT, P).transpose(1, 2, 0, 3)
    t = np.stack([ta, tb], axis=2)  # p h 2 k q
    return np.ascontiguousarray(t).reshape(P, HT * 2 * KT * P)


def _pack_wc(w, dtype, scale=1.0):
    """[HID, EMB] -> [P, HT*HT*P] with layout [p, d, h, q]."""
    t = np.ascontiguousarray(
        (w * scale).astype(dtype).reshape(HT, P, HT, P).transpose(1, 2, 0, 3)
    )
    return t.reshape(P, HT * HT * P)


def _pack_xt(xe, C, chunks, dtype):
    """tokens [n, EMB] (already scaled/cast) -> [P, KT*C] per-chunk blocks."""
    n = xe.shape[0]
    xt = np.zeros((KT, P, C), dtype=dtype)  # k p c
    xt[:, :, :n] = xe.reshape(n, KT, P).transpose(1, 2, 0)
    out = np.empty((P, KT * C), dtype=dtype)
    c0 = 0
    o = 0
    for cs in chunks:
        blk = xt[:, :, c0 : c0 + cs].transpose(1, 0, 2).reshape(P, KT * cs)
        out[:, o : o + KT * cs] = blk
        c0 += cs
        o += KT * cs
    return out


def kernel(x, w_gate, w1, w2, wc):
    trace = bool(int(os.environ.get("BASS_MOE_TRACE", "0")))
    if trace:
        _install_profile_shim()

    import concourse.bass_utils as bass_utils

    bass_utils.upload_artifacts = lambda tmpdir: f"local://{tmpdir}"

    x = np.asarray(x, dtype=np.float32)
    w_gate = np.asarray(w_gate, dtype=np.float32)
    w1 = np.asarray(w1, dtype=np.float32)
    w2 = np.asarray(w2, dtype=np.float32)
    wc = np.asarray(wc, dtype=np.float32)

    b, s, d = x.shape
    xf = x.reshape(-1, d)
    n = xf.shape[0]

    # ---- Router on host (float64: stable ranking + gate values) ----
    logits = xf.astype(np.float64) @ w_gate.astype(np.float64)
    mx = logits.max(axis=1, keepdims=True)
    p = np.exp(logits - mx)
    p /= p.sum(axis=1, keepdims=True)
    top = np.argpartition(-logits, TOPK, axis=1)[:, :TOPK]  # top-2 ids (unordered)

    sel_tok = []
    sel_gate = []
    flat_e = top.ravel()
    flat_t = np.repeat(np.arange(n), TOPK)
    order = np.argsort(flat_e, kind="stable")
    se, st = flat_e[order], flat_t[order]
    bounds = np.searchsorted(se, np.arange(E + 1))
    counts = np.diff(bounds)
    for e in range(E):
        toks = st[bounds[e] : bounds[e + 1]]
        sel_tok.append(toks)
        sel_gate.append(p[toks, e].astype(np.float32))

    # ---- Tier split: per expert, top-gate tokens -> bf16 (b0/b1 by count
    # slot), rest -> fp8 ----
    rank = np.argsort(-counts, kind="stable")
    slot_of = np.empty(E, dtype=int)
    slot_of[rank[:NCORES]] = 0
    slot_of[rank[NCORES:]] = 1
    nb_of = np.where(slot_of == 0, B0, B1)
    bf_tok, bf_gate, f8_tok, f8_gate = [], [], [], []
    for e in range(E):
        g = sel_gate[e]
        nb = min(int(nb_of[e]), len(g))
        o = np.argsort(-g, kind="stable")
        bf_tok.append(sel_tok[e][o[:nb]])
        bf_gate.append(g[o[:nb]])
        f8_tok.append(sel_tok[e][o[nb:]])
        f8_gate.append(g[o[nb:]])
    nf = np.array([len(t) for t in f8_tok])

    # bf16 slot assignment: biggest experts slot 0 (cap B0), rest slot 1
    slot_experts = [
        [int(rank[core]), int(rank[core + NCORES])] for core in range(NCORES)
    ]
    caps = [B0, B1]
    # fp8 slot assignment: biggest batches slot 0
    frank = np.argsort(-nf, kind="stable")
    fslot_experts = [
        [int(frank[core]), int(frank[core + NCORES])] for core in range(NCORES)
    ]
    fcaps = []
    for j in range(EPC):
        fmax = max(nf[fslot_experts[core][j]] for core in range(NCORES))
        fcaps.append(max(4, int(-(-fmax // 4) * 4)))

    # ---- Build per-core input maps ----
    xf_b = xf.astype(BF16)
    assert np.abs(xf * SX).max() < 239
    xf_8 = (xf * SX).astype(FP8)
    in_maps = []
    for core in range(NCORES):
        m = {}
        for j in range(EPC):
            e = slot_experts[core][j]
            C = caps[j]
            chunks = _token_chunks(C)
            toks = bf_tok[e]
            g = np.zeros((C,), dtype=np.float32)
            g[: len(toks)] = bf_gate[e]
            m[f"xt{j}"] = _pack_xt(xf_b[toks], C, chunks, BF16)
            m[f"g{j}"] = np.broadcast_to(g, (P, C)).copy()
            m[f"w12_{j}"] = _pack_w12(w1[e], w2[e], BF16)
            m[f"wc_{j}"] = _pack_wc(wc[e], BF16)
        for j in range(EPC):
            e = fslot_experts[core][j]
            Fc = fcaps[j]
            toks = f8_tok[e]
            g = np.zeros((Fc,), dtype=np.float32)
            g[: len(toks)] = f8_gate[e] / (SH * SWC)
            m[f"x8_{j}"] = _pack_xt(xf_8[toks], Fc, [Fc], FP8)
            m[f"g8_{j}"] = np.broadcast_to(g, (P, Fc)).copy()
            m[f"w128_{j}"] = _pack_w12(w1[e], w2[e], FP8, SW)
            m[f"wc8_{j}"] = _pack_wc(wc[e], FP8, SWC)
        in_maps.append(m)

    nc = _build_bass(caps, fcaps)
    res = bass_utils.run_bass_kernel_spmd(
        nc, in_maps, list(range(NCORES)), trace=trace
    )
    if trace:
        kernel.last_exec_time_ns = res.exec_time_ns
        kernel.last_trace = (
            res.instructions_and_trace[1] if res.instructions_and_trace else None
        )

    # ---- Scatter-add back to token order ----
    out = np.zeros((n, d), dtype=np.float32)
    for core in range(NCORES):
        for j in range(EPC):
            e = slot_experts[core][j]
            toks = bf_tok[e]
            C = caps[j]
            yt = (
                np.asarray(res.results[core][f"yt{j}"])
                .reshape(P, HT, C)
                .transpose(1, 0, 2)
                .reshape(EMB, C)
                .astype(np.float32)
            )
            out[toks] += yt[:, : len(toks)].T
        for j in range(EPC):
            e = fslot_experts[core][j]
            toks = f8_tok[e]
            Fc = fcaps[j]
            yt = (
                np.asarray(res.results[core][f"yt8_{j}"])
                .reshape(P, HT, Fc)
                .transpose(1, 0, 2)
                .reshape(EMB, Fc)
                .astype(np.float32)
            )
            out[toks] += yt[:, : len(toks)].T
    return out.reshape(b, s, d)


# revision 16
# speedup vs baseline: 1.2329x; 1.2329x over previous
"""MoE (top-2 of 16 experts, SwiGLU MLP) kernel for 8 Trainium2 NeuronCores.

Two-tier expert-parallel strategy:
  - Host router (fp64) gathers tokens per expert. Within each expert the
    highest-gate tokens run in bf16; the low-gate tail runs entirely in
    fp8-e4m3 with DoubleRow matmuls (2x PE rate). Low gates scale the
    fp8 error down so the global rel err stays ~1.6e-2 (< 2e-2 gate).
  - bf16 tier: 8 biggest experts get b0=824 tokens (slot 0), the rest
    b1=688 (slot 1) -- zero padding in the bf16 tier. fp8 batches are
    ranked by size into two slots with uniform caps.
  - Device math per batch:
        ht = silu(W1.T @ Xt) * (W2.T @ Xt);  yt = (WC.T @ ht) * gate
    fp8: x,w scaled on host (sx=0.6, sw=6, swc=512); silu descales via
    the activation input scale; h is cast to fp8 at scale sx*sw; the
    final descale is folded into the gate row.
  - Host: scatter-add per-batch outputs back to token order (fp32).
"""

import contextlib
import ctypes
import os
import sys
import types

sys.path.insert(0, "/opt/trn_rl_repo")

import numpy as np
import ml_dtypes

import concourse.bass as bass
import concourse.mybir as mybir
import concourse.tile as tile

BF16 = ml_dtypes.bfloat16
FP8 = ml_dtypes.float8_e4m3  # matches mybir.dt.float8e4 (max 240)
EMB = 1024
HID = 1024
E = 16
TOPK = 2
NCORES = 8
EPC = E // NCORES  # bf16 experts per core
P = 128
KT = EMB // P  # contraction tiles per GEMM (8)
HT = HID // P  # hidden/output row-blocks (8)

B0, B1 = 776, 640  # bf16 tokens per slot-0 / slot-1 expert
SX, SW, SWC = 0.6, 6.0, 512.0  # fp8 scales; h scale = SX*SW
SH = SX * SW


def _install_profile_shim():
    """Register the axon NTFF profiling hook (missing antenv.axon_hooks in
    this image) so run_bass_kernel_spmd(trace=True) can measure HW time."""
    if "antenv.axon_hooks" in sys.modules:
        return
    try:
        lib = ctypes.CDLL("/opt/axon/libaxon_pjrt.so")
        lib.axon_start_nrt_profile.argtypes = [
            ctypes.POINTER(ctypes.c_int64),
            ctypes.c_size_t,
        ]
        lib.axon_start_nrt_profile.restype = ctypes.c_int64
        lib.axon_stop_nrt_profile.argtypes = [ctypes.c_char_p]
        lib.axon_stop_nrt_profile.restype = ctypes.c_int64
    except Exception:
        return

    @contextlib.contextmanager
    def _hook(output_dir, device_ids):
        import jax

        jax.devices()
        ids = (
            (ctypes.c_int64 * len(device_ids))(*device_ids) if device_ids else None
        )
        rc = lib.axon_start_nrt_profile(ids, len(device_ids) if device_ids else 0)
        if rc != 0:
            raise RuntimeError(f"axon_start_nrt_profile rc={rc}")
        try:
            yield
        finally:
            n = lib.axon_stop_nrt_profile(str(output_dir).encode())
            print(f"profile: {n} file(s) written to {output_dir}")

    mod = types.ModuleType("antenv.axon_hooks")
    mod.get_axon_ntff_profile_hook = lambda: _hook
    mod.set_axon_ntff_profile_hook = lambda h: None
    sys.modules["antenv.axon_hooks"] = mod


def _split_multi_waits(nc):
    """This container's walrus only encodes one sem wait per CTRL-class
    instruction; hoist extra waits onto dedicated single-wait NoOps."""
    idx = 0
    for fn in nc.m.functions:
        for bb in fn.blocks:
            new = []
            for inst in bb.instructions:
                si = inst.sync_info
                if si is not None and len(si.on_wait) > 1:
                    waits = list(si.on_wait)
                    for w in waits[:-1]:
                        c = mybir.InstNoOp(name=f"wsplit-{idx}", ins=[], outs=[])
                        idx += 1
                        c.engine = inst.engine
                        c.sync_info = mybir.SyncInfo(on_wait=[w], on_update=[])
                        new.append(c)
                    si.on_wait = [waits[-1]]
                new.append(inst)
            bb.instructions = new


def _token_chunks(C):
    """Split C (multiple of 4) into a 320-col lead chunk (fast DMA -> PE
    starts early; paces the first h-pass to the weight feed) plus
    near-equal chunks <=512, multiples of 4."""
    lead = min(320, C)
    R = C - lead
    if R == 0:
        return [lead]
    n = max(1, -(-R // 512))
    base = R // n
    base -= base % 4
    sizes = [base] * n
    rem = R - base * n
    i = 0
    while rem > 0:
        add = min(4, rem)
        sizes[i] += add
        rem -= add
        i = (i + 1) % n
    return [lead] + [s for s in sizes if s > 0]


def _build_bass(caps, fcaps):
    F32 = mybir.dt.float32
    B16 = mybir.dt.bfloat16
    F8 = mybir.dt.float8e4
    DR = mybir.MatmulPerfMode.DoubleRow

    nc = bass.Bass()
    xt_d, g_d, w12_d, wc_d, yt_d = [], [], [], [], []
    for e in range(EPC):
        C = caps[e]
        xt_d.append(nc.declare_dram_parameter(f"xt{e}", [P, KT * C], B16, isOutput=False))
        g_d.append(nc.declare_dram_parameter(f"g{e}", [P, C], F32, isOutput=False))
        w12_d.append(
            nc.declare_dram_parameter(
                f"w12_{e}", [P, HT * 2 * KT * P], B16, isOutput=False
            )
        )
        wc_d.append(
            nc.declare_dram_parameter(f"wc_{e}", [P, HT * HT * P], B16, isOutput=False)
        )
        yt_d.append(nc.declare_dram_parameter(f"yt{e}", [P, HT * C], B16, isOutput=True))
    x8_d, g8_d, w128_d, wc8_d, yt8_d = [], [], [], [], []
    for f in range(EPC):
        Fc = fcaps[f]
        x8_d.append(nc.declare_dram_parameter(f"x8_{f}", [P, KT * Fc], F8, isOutput=False))
        g8_d.append(nc.declare_dram_parameter(f"g8_{f}", [P, Fc], F32, isOutput=False))
        w128_d.append(
            nc.declare_dram_parameter(
                f"w128_{f}", [P, HT * 2 * KT * P], F8, isOutput=False
            )
        )
        wc8_d.append(
            nc.declare_dram_parameter(f"wc8_{f}", [P, HT * HT * P], F8, isOutput=False)
        )
        yt8_d.append(
            nc.declare_dram_parameter(f"yt8_{f}", [P, HT * Fc], B16, isOutput=True)
        )

    with tile.TileContext(nc) as tc:
        with (
            tc.tile_pool(name="xt", bufs=4) as xt_pool,
            tc.tile_pool(name="x8", bufs=2) as x8_pool,
            tc.tile_pool(name="ht", bufs=2) as ht_pool,
            tc.tile_pool(name="h8", bufs=2) as h8_pool,
            tc.tile_pool(name="g", bufs=2) as g_pool,
            tc.tile_pool(name="g8", bufs=2) as g8_pool,
            tc.tile_pool(name="w12", bufs=10) as w12_pool,
            tc.tile_pool(name="w128", bufs=17) as w128_pool,
            tc.tile_pool(name="wc", bufs=8) as wc_pool,
            tc.tile_pool(name="wc8", bufs=16) as wc8_pool,
            tc.tile_pool(name="s", bufs=3) as s_pool,
            tc.tile_pool(name="y", bufs=3) as y_pool,
            tc.tile_pool(name="psA", bufs=2, space="PSUM") as psA,
            tc.tile_pool(name="psB", bufs=3, space="PSUM") as psB,
            tc.tile_pool(name="psW", bufs=1, space="PSUM") as psW,
        ):
            # PE p-state warm-up during the initial DMA wait.
            warm_in = s_pool.tile([P, 256], mybir.dt.bfloat16, tag="warm")
            nc.vector.memset(warm_in[:], 0.0)
            warm_ps = psW.tile([P, 256], mybir.dt.float32, tag="warmps")
            for _ in range(20):
                nc.tensor.matmul(
                    warm_ps[:], warm_in[:, 0:P], warm_in[:], start=True, stop=True
                )

            all_chunks = [_token_chunks(caps[e]) for e in range(EPC)]
            w12t = [[None] * HT for _ in range(EPC)]
            w128t = [[None] * HT for _ in range(EPC)]
            xt_t = [[] for _ in range(EPC)]
            x8_t = [None] * EPC
            g_sb = [None] * EPC
            g8_sb = [None] * EPC
            WB = 2 * KT * P  # w12 elements per h-block

            # ---- Hoisted input DMA issues (in-order sync engine: these
            # must never queue behind semaphore-gated y-DMA issues).
            # Priority: e0 weights h0 + lead chunk, e0 rest, e1 critical
            # prefix, fp8 f0 inputs, gates. The remainder is issued after
            # e0's phase A (pool buffers free up by then).
            def issue_w12(e, h, eng=None):
                w12t[e][h] = w12_pool.tile(
                    [P, 2, KT, P], B16, tag="w12", name=f"w12_{e}_{h}"
                )
                (eng or nc.sync).dma_start(
                    w12t[e][h][:],
                    w12_d[e][:, h * WB : (h + 1) * WB].rearrange(
                        "p (t k q) -> p t k q", t=2, k=KT
                    ),
                )

            def issue_xt(e, ci, c0, cs, eng=None):
                t = xt_pool.tile([P, KT, cs], B16, tag="xt", name=f"xt_{e}_{ci}")
                xt_t[e].append(t)
                (eng or nc.sync).dma_start(
                    t[:],
                    xt_d[e][:, KT * c0 : KT * (c0 + cs)].rearrange(
                        "p (k c) -> p k c", k=KT
                    ),
                )

            def issue_w128(f, h):
                w128t[f][h] = w128_pool.tile(
                    [P, 2, KT, P], F8, tag="w128", name=f"w128_{f}_{h}"
                )
                nc.sync.dma_start(
                    w128t[f][h][:],
                    w128_d[f][:, h * WB : (h + 1) * WB].rearrange(
                        "p (t k q) -> p t k q", t=2, k=KT
                    ),
                )

            def issue_x8(f):
                x8_t[f] = x8_pool.tile(
                    [P, KT, fcaps[f]], F8, tag="x8", name=f"x8_{f}"
                )
                nc.sync.dma_start(
                    x8_t[f][:],
                    x8_d[f].rearrange("p (k c) -> p k c", k=KT),
                )

            # e0 critical path -- first issues fan out across engines so
            # the serial ~0.7us-per-issue cost is paid in parallel
            ch0 = all_chunks[0]
            issue_w12(0, 0)                      # sync
            issue_xt(0, 0, 0, ch0[0], nc.scalar)
            issue_w12(0, 1)
            issue_w12(0, 2, nc.scalar)
            issue_w12(0, 3)                      # sync
            issue_w12(0, 4)
            issue_w12(0, 5)
            issue_w12(0, 6)                      # sync
            issue_w12(0, 7)
            c0 = ch0[0]
            for ci in range(1, len(ch0)):
                issue_xt(0, ci, c0, ch0[ci])
                c0 += ch0[ci]
            # gates + e0's wc (needed when e0 phase B starts)
            def issue_g(e):
                g_sb[e] = g_pool.tile([P, caps[e]], F32, tag="g", name=f"g_{e}")
                nc.sync.dma_start(g_sb[e][:], g_d[e][:])

            def issue_g8(f):
                g8_sb[f] = g8_pool.tile(
                    [P, fcaps[f]], F32, tag="g8", name=f"g8_{f}"
                )
                nc.sync.dma_start(g8_sb[f][:], g8_d[f][:])

            def phase_a_bf16(e):
                C = caps[e]
                chunks = all_chunks[e]
                ht_sb = ht_pool.tile([P, HT, C], B16, tag="ht", name=f"ht_{e}")
                c0 = 0
                for ci, cs in enumerate(chunks):
                    for h in range(HT):
                        ps1 = psA.tile([P, cs], mybir.dt.float32, tag="ps1")
                        ps2 = psA.tile([P, cs], mybir.dt.float32, tag="ps2")
                        for k in range(KT):
                            nc.tensor.matmul(
                                ps1[:],
                                w12t[e][h][:, 0, k, :],
                                xt_t[e][ci][:, k, :],
                                start=(k == 0),
                                stop=(k == KT - 1),
                            )
                        for k in range(KT):
                            nc.tensor.matmul(
                                ps2[:],
                                w12t[e][h][:, 1, k, :],
                                xt_t[e][ci][:, k, :],
                                start=(k == 0),
                                stop=(k == KT - 1),
                            )
                        s_sb = s_pool.tile([P, 512], mybir.dt.float32, tag="s")
                        nc.scalar.activation(
                            s_sb[:, :cs],
                            ps1[:],
                            mybir.ActivationFunctionType.Silu,
                        )
                        nc.vector.tensor_mul(
                            ht_sb[:, h, c0 : c0 + cs], s_sb[:, :cs], ps2[:]
                        )
                    c0 += cs
                return ht_sb

            def issue_wcts(e):
                wcts = []
                for d in range(HT):
                    wct = wc_pool.tile(
                        [P, HT, P], B16, tag="wc", name=f"wc_{e}_{d}"
                    )
                    wcts.append(wct)
                    nc.sync.dma_start(
                        wct[:],
                        wc_d[e][:, d * HT * P : (d + 1) * HT * P].rearrange(
                            "p (h q) -> p h q", h=HT
                        ),
                    )
                return wcts

            def issue_wc8s(f):
                wcts = []
                for d in range(HT):
                    wct = wc8_pool.tile(
                        [P, HT, P], F8, tag="wc8", name=f"wc8_{f}_{d}"
                    )
                    wcts.append(wct)
                    nc.sync.dma_start(
                        wct[:],
                        wc8_d[f][:, d * HT * P : (d + 1) * HT * P].rearrange(
                            "p (h q) -> p h q", h=HT
                        ),
                    )
                return wcts

            def phase_b_bf16(e, ht_sb, wcts):
                chunks = all_chunks[e]
                offs = []
                c0 = 0
                for cs in chunks:
                    offs.append((c0, cs))
                    c0 += cs
                # natural order: lead chunk first (its ht was finished
                # earliest in phase A, so B never waits on the DVE tail)
                yt_view = yt_d[e].rearrange("p (d c) -> p d c", d=HT)
                for d in range(HT):
                    for c0, cs in offs:
                        psy = psB.tile([P, cs], mybir.dt.float32, tag="psy")
                        for h in range(HT):
                            nc.tensor.matmul(
                                psy[:],
                                wcts[d][:, h, :],
                                ht_sb[:, h, c0 : c0 + cs],
                                start=(h == 0),
                                stop=(h == HT - 1),
                            )
                        y_sb = y_pool.tile([P, 512], B16, tag="y")
                        nc.vector.tensor_mul(
                            y_sb[:, :cs], psy[:], g_sb[e][:, c0 : c0 + cs]
                        )
                        nc.sync.dma_start(
                            yt_view[:, d, c0 : c0 + cs],
                            y_sb[:, :cs],
                        )

            def fp8_phase_a(f):
                Fc = fcaps[f]
                h8_sb = h8_pool.tile([P, HT, Fc], F8, tag="h8", name=f"h8_{f}")
                # Phase A: 4 DoubleRow matmuls per GEMM per h-block
                for h in range(HT):
                    ps1 = psA.tile([P, Fc], mybir.dt.float32, tag="ps1")
                    ps2 = psA.tile([P, Fc], mybir.dt.float32, tag="ps2")
                    for kp in range(KT // 2):
                        nc.tensor.matmul(
                            ps1[:],
                            w128t[f][h][:, 0, 2 * kp : 2 * kp + 2, :],
                            x8_t[f][:, 2 * kp : 2 * kp + 2, :],
                            start=(kp == 0),
                            stop=(kp == KT // 2 - 1),
                            perf_mode=DR,
                        )
                    for kp in range(KT // 2):
                        nc.tensor.matmul(
                            ps2[:],
                            w128t[f][h][:, 1, 2 * kp : 2 * kp + 2, :],
                            x8_t[f][:, 2 * kp : 2 * kp + 2, :],
                            start=(kp == 0),
                            stop=(kp == KT // 2 - 1),
                            perf_mode=DR,
                        )
                    s_sb = s_pool.tile([P, 512], mybir.dt.float32, tag="s")
                    nc.scalar.activation(
                        s_sb[:, :Fc],
                        ps1[:],
                        mybir.ActivationFunctionType.Silu,
                        scale=1.0 / (SX * SW),
                    )
                    # h8 = silu(a) * b_psum  -> scale sx*sw, cast to fp8
                    nc.vector.tensor_mul(h8_sb[:, h, :], s_sb[:, :Fc], ps2[:])
                return h8_sb

            def fp8_phase_b(f, h8_sb, wcts):
                Fc = fcaps[f]
                yt_view = yt8_d[f].rearrange("p (d c) -> p d c", d=HT)
                for d in range(HT):
                    psy = psB.tile([P, Fc], mybir.dt.float32, tag="psy")
                    for hp in range(HT // 2):
                        nc.tensor.matmul(
                            psy[:],
                            wcts[d][:, 2 * hp : 2 * hp + 2, :],
                            h8_sb[:, 2 * hp : 2 * hp + 2, :],
                            start=(hp == 0),
                            stop=(hp == HT // 2 - 1),
                            perf_mode=DR,
                        )
                    y_sb = y_pool.tile([P, 512], B16, tag="y")
                    nc.vector.tensor_mul(y_sb[:, :Fc], psy[:], g8_sb[f][:])
                    nc.sync.dma_start(yt_view[:, d, :], y_sb[:, :Fc])

            # remaining hoisted issues, in consumption order
            issue_g(0)
            wcts0 = issue_wcts(0)
            issue_w12(1, 0)
            issue_xt(1, 0, 0, all_chunks[1][0])
            issue_w12(1, 1)
            for h in range(HT):
                issue_w128(0, h)
            issue_x8(0)
            issue_g(1)
            issue_g8(0)
            issue_g8(1)

            # ---- Compute: e0 bf16, e1 bf16, then fp8 A/A/B/B ----
            ht0 = phase_a_bf16(0)
            # deferred input issues (buffers free during e0 phase A)
            for h in range(2, HT):
                issue_w12(1, h)
            c0 = all_chunks[1][0]
            for ci in range(1, len(all_chunks[1])):
                issue_xt(1, ci, c0, all_chunks[1][ci])
                c0 += all_chunks[1][ci]
            for h in range(HT):
                issue_w128(1, h)
            issue_x8(1)
            phase_b_bf16(0, ht0, wcts0)
            ht1 = phase_a_bf16(1)
            wcts1 = issue_wcts(1)
            wc8s0 = issue_wc8s(0)
            wc8s1 = issue_wc8s(1)
            phase_b_bf16(1, ht1, wcts1)
            h8_0 = fp8_phase_a(0)
            h8_1 = fp8_phase_a(1)
            fp8_phase_b(0, h8_0, wc8s0)
            fp8_phase_b(1, h8_1, wc8s1)

    _split_multi_waits(nc)
    return nc


def _pack_w12(wa, wb, dtype, scale=1.0):
    """[EMB, HID] x2 -> [P, HT*2*KT*P] with layout [p, h, {a,b}, k, q]."""
    ta = (wa * scale).astype(dtype).reshape(KT, P, HT, P).transpose(1, 2, 0, 3)
    tb = (wb * scale).astype(dtype).reshape(KT, P, HT, P).transpose(1, 2, 0, 3)
    t = np.stack([ta, tb], axis=2)  # p h 2 k q
    return np.ascontiguousarray(t).reshape(P, HT * 2 * KT * P)


def _pack_wc(w, dtype, scale=1.0):
    """[HID, EMB] -> [P, HT*HT*P] with layout [p, d, h, q]."""
    t = np.ascontiguousarray(
        (w * scale).astype(dtype).reshape(HT, P, HT, P).transpose(1, 2, 0, 3)
    )
    return t.reshape(P, HT * HT * P)


def _pack_xt(xe, C, chunks, dtype):
    """tokens [n, EMB] (already scaled/cast) -> [P, KT*C] per-chunk blocks."""
    n = xe.shape[0]
    xt = np.zeros((KT, P, C), dtype=dtype)  # k p c
    xt[:, :, :n] = xe.reshape(n, KT, P).transpose(1, 2, 0)
    out = np.empty((P, KT * C), dtype=dtype)
    c0 = 0
    o = 0
    for cs in chunks:
        blk = xt[:, :, c0 : c0 + cs].transpose(1, 0, 2).reshape(P, KT * cs)
        out[:, o : o + KT * cs] = blk
        c0 += cs
        o += KT * cs
    return out


def kernel(x, w_gate, w1, w2, wc):
    trace = bool(int(os.environ.get("BASS_MOE_TRACE", "0")))
    if trace:
        _install_profile_shim()

    import concourse.bass_utils as bass_utils

    bass_utils.upload_artifacts = lambda tmpdir: f"local://{tmpdir}"

    x = np.asarray(x, dtype=np.float32)
    w_gate = np.asarray(w_gate, dtype=np.float32)
    w1 = np.asarray(w1, dtype=np.float32)
    w2 = np.asarray(w2, dtype=np.float32)
    wc = np.asarray(wc, dtype=np.float32)

    b, s, d = x.shape
    xf = x.reshape(-1, d)
    n = xf.shape[0]

    # ---- Router on host (float64: stable ranking + gate values) ----
    logits = xf.astype(np.float64) @ w_gate.astype(np.float64)
    mx = logits.max(axis=1, keepdims=True)
    p = np.exp(logits - mx)
    p /= p.sum(axis=1, keepdims=True)
    top = np.argpartition(-logits, TOPK, axis=1)[:, :TOPK]  # top-2 ids (unordered)

    sel_tok = []
    sel_gate = []
    flat_e = top.ravel()
    flat_t = np.repeat(np.arange(n), TOPK)
    order = np.argsort(flat_e, kind="stable")
    se, st = flat_e[order], flat_t[order]
    bounds = np.searchsorted(se, np.arange(E + 1))
    counts = np.diff(bounds)
    for e in range(E):
        toks = st[bounds[e] : bounds[e + 1]]
        sel_tok.append(toks)
        sel_gate.append(p[toks, e].astype(np.float32))

    # ---- Tier split: per expert, top-gate tokens -> bf16 (b0/b1 by count
    # slot), rest -> fp8 ----
    rank = np.argsort(-counts, kind="stable")
    slot_of = np.empty(E, dtype=int)
    slot_of[rank[:NCORES]] = 0
    slot_of[rank[NCORES:]] = 1
    nb_of = np.where(slot_of == 0, B0, B1)
    bf_tok, bf_gate, f8_tok, f8_gate = [], [], [], []
    for e in range(E):
        g = sel_gate[e]
        nb = min(int(nb_of[e]), len(g))
        o = np.argsort(-g, kind="stable")
        bf_tok.append(sel_tok[e][o[:nb]])
        bf_gate.append(g[o[:nb]])
        f8_tok.append(sel_tok[e][o[nb:]])
        f8_gate.append(g[o[nb:]])
    nf = np.array([len(t) for t in f8_tok])

    # bf16 slot assignment: biggest experts slot 0 (cap B0), rest slot 1
    slot_experts = [
        [int(rank[core]), int(rank[core + NCORES])] for core in range(NCORES)
    ]
    caps = [B0, B1]
    # fp8 slot assignment: biggest batches slot 0
    frank = np.argsort(-nf, kind="stable")
    fslot_experts = [
        [int(frank[core]), int(frank[core + NCORES])] for core in range(NCORES)
    ]
    fcaps = []
    for j in range(EPC):
        fmax = max(nf[fslot_experts[core][j]] for core in range(NCORES))
        fcaps.append(max(4, int(-(-fmax // 4) * 4)))

    # ---- Build per-core input maps ----
    xf_b = xf.astype(BF16)
    assert np.abs(xf * SX).max() < 239
    xf_8 = (xf * SX).astype(FP8)
    in_maps = []
    for core in range(NCORES):
        m = {}
        for j in range(EPC):
            e = slot_experts[core][j]
            C = caps[j]
            chunks = _token_chunks(C)
            toks = bf_tok[e]
            g = np.zeros((C,), dtype=np.float32)
            g[: len(toks)] = bf_gate[e]
            m[f"xt{j}"] = _pack_xt(xf_b[toks], C, chunks, BF16)
            m[f"g{j}"] = np.broadcast_to(g, (P, C)).copy()
            m[f"w12_{j}"] = _pack_w12(w1[e], w2[e], BF16)
            m[f"wc_{j}"] = _pack_wc(wc[e], BF16)
        for j in range(EPC):
            e = fslot_experts[core][j]
            Fc = fcaps[j]
            toks = f8_tok[e]
            g = np.zeros((Fc,), dtype=np.float32)
            g[: len(toks)] = f8_gate[e] / (SH * SWC)
            m[f"x8_{j}"] = _pack_xt(xf_8[toks], Fc, [Fc], FP8)
            m[f"g8_{j}"] = np.broadcast_to(g, (P, Fc)).copy()
            m[f"w128_{j}"] = _pack_w12(w1[e], w2[e], FP8, SW)
            m[f"wc8_{j}"] = _pack_wc(wc[e], FP8, SWC)
        in_maps.append(m)

    nc = _build_bass(caps, fcaps)
    res = bass_utils.run_bass_kernel_spmd(
        nc, in_maps, list(range(NCORES)), trace=trace
    )
    if trace:
        kernel.last_exec_time_ns = res.exec_time_ns
        kernel.last_trace = (
            res.instructions_and_trace[1] if res.instructions_and_trace else None
        )

    # ---- Scatter-add back to token order ----
    out = np.zeros((n, d), dtype=np.float32)
    for core in range(NCORES):
        for j in range(EPC):
            e = slot_experts[core][j]
            toks = bf_tok[e]
            C = caps[j]
            yt = (
                np.asarray(res.results[core][f"yt{j}"])
                .reshape(P, HT, C)
                .transpose(1, 0, 2)
                .reshape(EMB, C)
                .astype(np.float32)
            )
            out[toks] += yt[:, : len(toks)].T
        for j in range(EPC):
            e = fslot_experts[core][j]
            toks = f8_tok[e]
            Fc = fcaps[j]
            yt = (
                np.asarray(res.results[core][f"yt8_{j}"])
                .reshape(P, HT, Fc)
                .transpose(1, 0, 2)
                .reshape(EMB, Fc)
                .astype(np.float32)
            )
            out[toks] += yt[:, : len(toks)].T
    return out.reshape(b, s, d)
